# revision 35
# baseline (speedup 1.0000x reference)
"""Trainium2 Bass kernel for a dense pre-LN transformer block.

B=4, T=1024, C=1024, H=16 heads (head_size 64).

Distribution over the 8 NeuronCores (two SPMD launches, host-side
reshuffle between them):

  Launch A (attention, batch x head-half parallel): core c handles
  batch b=c//2 and heads h0=(c%2)*8 .. h0+8. Each core runs LN1 on its
  own 1024 rows only (no cross-core redundancy), projects q/k/v for its
  8 heads, computes causal softmax attention and writes the normalized
  per-head outputs cat [1024, 512] (head-major columns) in bf16.
  NOTE the reference computes scores as k @ q^T (roles of q/k swapped
  vs standard attention) — handled by using k rows as the "queries".

  Host: reassemble cat_all [B*T, C] from the per-core column halves.

  Launch B (FFN, row-parallel): core c runs proj(cat)+residual, LN2,
  W1/PReLU/W2 + residual on rows [512c, 512(c+1)).

Matmul operands are bf16 throughout (2x DMA saving, FWL fast weight
loads); accumulation stays fp32 in PSUM. LN statistics are computed in
fp32. Residual adds are fp32.
"""

from contextlib import ExitStack

import numpy as np
import ml_dtypes

import concourse.bass as bass
import concourse.tile as tile
from concourse import bacc, mybir
from concourse.bass_utils import run_bass_kernel_spmd
from concourse.masks import make_identity

F32 = mybir.dt.float32
F32R = mybir.dt.float32r
BF16 = mybir.dt.bfloat16
AF = mybir.ActivationFunctionType
ALU = mybir.AluOpType

B, T, C, H, HS = 4, 1024, 1024, 16, 64
NCORES = 8
EPS = 1e-5
SCALE = float(C) ** -0.5  # 1/32, folded into the softmax exp
NEG = -1e30

NTB = T // 128   # 8 token blocks per batch
NCC = C // 128   # 8 channel chunks
HPC = 8          # heads per core
NHP = HPC // 2   # head pairs per core


# --------------------------------------------------------------------------
# kernel A: attention, one batch + 8 heads per core
# --------------------------------------------------------------------------

def _attn_body(ctx, tc, x, wq, wk, wv, lnw, lnb, catout):
    """Transposed-scores attention: scoresT[s,t] with s on partitions.

    softmax denominator comes from an appended ones-column in v (av psum
    column 64), normalization is a per-partition scale on the av output.
    """
    nc = tc.nc

    const = ctx.enter_context(tc.tile_pool(name="const", bufs=1))
    scratch = const.tile([128, 128], F32)
    make_identity(nc, scratch)
    identb = const.tile([128, 128], BF16)
    nc.vector.tensor_copy(out=identb, in_=scratch)
    # transposed causal mask for diagonal blocks: keep s<=t (cols>=rows)
    trilT = const.tile([128, 128], F32)
    nc.gpsimd.memset(trilT, 0.0)
    nc.gpsimd.affine_select(
        out=trilT, in_=trilT, compare_op=ALU.is_ge, fill=NEG, base=0,
        pattern=[[1, 128]], channel_multiplier=-1)
    ones8 = const.tile([128, NTB], BF16)
    nc.vector.memset(ones8, 1.0)
    eps_t = const.tile([128, 1], F32)
    nc.vector.memset(eps_t, EPS)

    wq_sb = const.tile([128, NCC, 512], BF16, tag="wq")
    wk_sb = const.tile([128, NCC, 512], BF16, tag="wk")
    wv_sb = const.tile([128, NCC, 512], BF16, tag="wv")
    general_ln = lnw is not None
    if general_ln:
        lnw_bc = const.tile([128, C], F32, tag="lnw")
        lnb_bc = const.tile([128, C], F32, tag="lnb")
        nc.sync.dma_start(
            out=lnw_bc,
            in_=bass.AP(tensor=lnw.tensor, offset=lnw.offset,
                        ap=[[0, 128]] + list(lnw.ap)))
        nc.sync.dma_start(
            out=lnb_bc,
            in_=bass.AP(tensor=lnb.tensor, offset=lnb.offset,
                        ap=[[0, 128]] + list(lnb.ap)))

    xp = ctx.enter_context(tc.tile_pool(name="xp", bufs=5))
    hp = ctx.enter_context(tc.tile_pool(name="hp", bufs=3))
    hTp = ctx.enter_context(tc.tile_pool(name="hTp", bufs=1))
    stat = ctx.enter_context(tc.tile_pool(name="stat", bufs=4))
    qkvp = ctx.enter_context(tc.tile_pool(name="qkvp", bufs=2))
    v2p = ctx.enter_context(tc.tile_pool(name="v2p", bufs=2))
    epl = ctx.enter_context(tc.tile_pool(name="epl", bufs=3))
    catp = ctx.enter_context(tc.tile_pool(name="catp", bufs=3))
    tmpp = ctx.enter_context(tc.tile_pool(name="tmpp", bufs=2))

    # PSUM banks: PST 2x1 + mm 4x1 + PSAV 2x1 = 8
    PST = ctx.enter_context(tc.tile_pool(name="pst", bufs=1, space="PSUM"))
    PSM = ctx.enter_context(tc.tile_pool(name="psm", bufs=4, space="PSUM"))
    PSAV = ctx.enter_context(tc.tile_pool(name="psav", bufs=3, space="PSUM"))

    # ---- LN1: groups of 2 token tiles; rstd = 1/sqrt(var+eps) via the
    # Sqrt table + DVE reciprocal (avoids Ln/Exp table thrash) ----
    h_tiles = []
    for g in range(4):
        mvs = stat.tile([128, 2, 2], F32, tag="mvs", name=f"mvs_{g}")
        rstd = stat.tile([128, 2], F32, tag="rstd", name=f"rstd_{g}")
        sqv = stat.tile([128, 2], F32, tag="sqv", name=f"sqv_{g}")
        xts = []
        for j in range(2):
            i = g * 2 + j
            xt = xp.tile([128, C], F32, tag="x", name=f"x_{i}")
            deng = nc.gpsimd if i % 2 else nc.sync
            deng.dma_start(out=xt, in_=x[i * 128:(i + 1) * 128, :])
            st = stat.tile([128, 2, 6], F32, tag="bn", name=f"bn_{i}")
            for k in range(2):
                nc.vector.bn_stats(out=st[:, k, :],
                                   in_=xt[:, k * 512:(k + 1) * 512])
            nc.vector.bn_aggr(out=mvs[:, j, :], in_=st)
            xts.append(xt)
        nc.scalar.activation(out=sqv, in_=mvs[:, :, 1], func=AF.Sqrt,
                             bias=eps_t)
        nc.vector.reciprocal(out=rstd, in_=sqv)
        for j in range(2):
            i = g * 2 + j
            ht = hp.tile([128, C], BF16, tag="h", name=f"h_{i}")
            if general_ln:
                t32 = tmpp.tile([128, C], F32, tag="t32", name=f"t32_{i}")
                nc.vector.tensor_scalar(
                    out=t32, in0=xts[j], scalar1=mvs[:, j, 0:1],
                    scalar2=rstd[:, j:j + 1], op0=ALU.subtract, op1=ALU.mult)
                nc.vector.tensor_mul(out=t32, in0=t32, in1=lnw_bc)
                nc.vector.tensor_add(out=ht, in0=t32, in1=lnb_bc)
            else:
                nc.vector.tensor_scalar(
                    out=ht, in0=xts[j], scalar1=mvs[:, j, 0:1],
                    scalar2=rstd[:, j:j + 1], op0=ALU.subtract, op1=ALU.mult)
            h_tiles.append((i, ht))

    # weight DMAs on the gpsimd queue: stream in parallel with the x
    # tiles on the sync queue
    nc.gpsimd.dma_start(out=wq_sb,
                        in_=wq.rearrange("(cc p) d -> p cc d", p=128))
    nc.gpsimd.dma_start(out=wk_sb,
                        in_=wk.rearrange("(cc p) d -> p cc d", p=128))
    nc.gpsimd.dma_start(out=wv_sb,
                        in_=wv.rearrange("(cc p) d -> p cc d", p=128))

    # ---- transpose h -> hT: tile-outer so PE starts on first LN'd tile ----
    hT = hTp.tile([128, NCC, T], BF16, tag="hT")
    for i, ht in h_tiles:
        pt = PST.tile([128, C], BF16, tag="tr", name=f"pt_{i}")
        for cc in range(NCC):
            nc.tensor.transpose(
                pt[:, cc * 128:(cc + 1) * 128],
                ht[:, cc * 128:(cc + 1) * 128], identb)
        src = pt.rearrange("p (cc q) -> p cc q", q=128)
        nc.vector.tensor_copy(out=hT[:, :, i * 128:(i + 1) * 128], in_=src)

    # ---- per-pair q/k/v + v2 (interleaved into the softmax pipeline so
    # the PE stays dense while the scalar engine runs exp) ----
    qTs, kTs, v2s = {}, {}, {}

    def qkv_block(p):
        dsl = slice(p * 128, (p + 1) * 128)
        qT = qkvp.tile([128, T], BF16, tag="qT", name=f"qT{p}")
        kT = qkvp.tile([128, T], BF16, tag="kT", name=f"kT{p}")
        vT = qkvp.tile([128, T], BF16, tag="vT", name=f"vT{p}")
        for wi, (wsb, dst) in enumerate(
                ((wq_sb, qT), (wk_sb, kT), (wv_sb, vT))):
            for tch in range(T // 512):
                tsl = slice(tch * 512, (tch + 1) * 512)
                pq = PSM.tile([128, 512], F32, tag="mm",
                              name=f"p_{p}_{wi}_{tch}")
                for cc in range(NCC):
                    nc.tensor.matmul(pq, wsb[:, cc, dsl], hT[:, cc, tsl],
                                     start=(cc == 0), stop=(cc == NCC - 1))
                nc.vector.tensor_copy(out=dst[:, tsl], in_=pq)
        v2 = v2p.tile([128, NTB, 132], BF16, tag="v2", name=f"v2_{p}")
        nc.vector.memset(v2, 0.0)
        for i in range(NTB):
            nc.vector.tensor_copy(out=v2[:, i, 64:65], in_=ones8[:, i:i + 1])
            nc.vector.tensor_copy(out=v2[:, i, 130:131], in_=ones8[:, i:i + 1])
        pv = PST.tile([128, T], BF16, tag="tr", name=f"pv_{p}")
        for i in range(NTB):
            nc.tensor.transpose(
                pv[:, i * 128:(i + 1) * 128],
                vT[:, i * 128:(i + 1) * 128], identb)
        for i in range(NTB):
            nc.vector.tensor_copy(out=v2[:, i, 0:64],
                                  in_=pv[:, i * 128:i * 128 + 64])
            nc.vector.tensor_copy(out=v2[:, i, 66:130],
                                  in_=pv[:, i * 128 + 64:(i + 1) * 128])
        qTs[p], kTs[p], v2s[p] = qT, kT, v2

    # ---- attention per head pair (scores row-group concurrent) ----
    # Output per head: catT_h [66, T]: rows 0:64 = unnormalized att@v in
    # [d, t] layout, row 64 = softmax denominator (host divides).
    def scores_block(p):
        qT, kT = qTs[p], kTs[p]
        es = {0: [], 1: []}  # h2 -> list of e_sc
        for sc in range(NTB):
            W = (NTB - sc) * 128  # t columns: blocks sc..7
            e2 = [epl.tile([128, W], BF16, tag=f"e{sc}_{h2}",
                           name=f"e_{p}_{sc}_{h2}") for h2 in (0, 1)]
            n0 = 0
            while n0 < W:
                n1 = min(n0 + 512, W)
                ps2 = []
                for h2 in (0, 1):
                    off = h2 * 64
                    pss = PSM.tile([128, 512], F32, tag="mm",
                                   name=f"pss_{p}_{sc}_{n0}_{h2}")
                    nc.tensor.matmul(
                        pss[:, 0:n1 - n0],
                        qT[off:off + 64, sc * 128:(sc + 1) * 128],
                        kT[off:off + 64, sc * 128 + n0:sc * 128 + n1],
                        start=True, stop=True)
                    ps2.append(pss)
                for h2 in (0, 1):
                    if n0 == 0:
                        nc.vector.tensor_add(out=ps2[h2][:, 0:128],
                                             in0=ps2[h2][:, 0:128],
                                             in1=trilT)
                    nc.scalar.activation(out=e2[h2][:, n0:n1],
                                         in_=ps2[h2][:, 0:n1 - n0],
                                         func=AF.Exp, scale=SCALE)
                n0 = n1
            for h2 in (0, 1):
                es[h2].append(e2[h2])
        return es

    def av_block(p, es):
        # swapped av: stationary v2 [s,66], moving e [s,t], out [66,t]
        v2 = v2s[p]
        for h2 in (0, 1):
            h = p * 2 + h2
            v0 = h2 * 66
            catTh = catp.tile([66, T], BF16, tag="catTh", name=f"catTh_{h}")
            for half in range(2):
                t0 = half * 512
                po = PSAV.tile([66, 512], F32, tag="av",
                               name=f"po_{h}_{half}")
                scs = [sc for sc in range(NTB) if sc * 128 < t0 + 512]
                for sc in scs:
                    st0 = sc * 128
                    lo = max(t0, st0)
                    nc.tensor.matmul(
                        po[:, lo - t0:512],
                        v2[:, sc, v0:v0 + 66],
                        es[h2][sc][:, lo - st0:t0 + 512 - st0],
                        start=(sc == 0), stop=(sc == scs[-1]))
                nc.vector.tensor_copy(out=catTh[:, t0:t0 + 512], in_=po)
            nc.sync.dma_start(out=catout[h * 65:(h + 1) * 65, :],
                              in_=catTh[0:65, :])

    # software pipeline: qkv(p)+scores(p) issue ahead of av(p-1), so the
    # PE FIFO never stalls waiting for the scalar exp
    pair_es = {}
    for p in range(NHP + 1):
        if p < NHP:
            qkv_block(p)
            pair_es[p] = scores_block(p)
        if p > 0:
            av_block(p - 1, pair_es.pop(p - 1))


def _build_attn(general_ln: bool):
    nc = bacc.Bacc("TRN2", target_bir_lowering=False, debug=False)
    x = nc.dram_tensor("x", [T, C], F32, kind="ExternalInput").ap()
    wq = nc.dram_tensor("wq", [C, 512], BF16, kind="ExternalInput").ap()
    wk = nc.dram_tensor("wk", [C, 512], BF16, kind="ExternalInput").ap()
    wv = nc.dram_tensor("wv", [C, 512], BF16, kind="ExternalInput").ap()
    lnw = lnb = None
    if general_ln:
        lnw = nc.dram_tensor("lnw", [C], F32, kind="ExternalInput").ap()
        lnb = nc.dram_tensor("lnb", [C], F32, kind="ExternalInput").ap()
    catout = nc.dram_tensor("catout", [HPC * 65, T], BF16,
                            kind="ExternalOutput").ap()
    with tile.TileContext(nc) as tc:
        with ExitStack() as ctx:
            _attn_body(ctx, tc, x, wq, wk, wv, lnw, lnb, catout)
    nc.compile()
    return nc


# --------------------------------------------------------------------------
# kernel B: FFN, 512 rows per core
# --------------------------------------------------------------------------

RPC = (B * T) // NCORES  # 512 rows per core
NRB = RPC // 128         # 4 row blocks
NHID = 4 * C // 128      # 32 hidden chunks


def _ffn_body(ctx, tc, xr, cat, wo, w1, w2, bo, b1, ln2w, ln2b, b2,
              alpha, out):
    """Per-core rows: proj = cat @ Wo (+bo); x2 = x + proj; LN2 + FFN."""
    nc = tc.nc
    general_ln = ln2w is not None

    const = ctx.enter_context(tc.tile_pool(name="const", bufs=1))
    scratch = const.tile([128, 128], F32)
    make_identity(nc, scratch)
    identb = const.tile([128, 128], BF16)
    nc.vector.tensor_copy(out=identb, in_=scratch)
    eps_t = const.tile([128, 1], F32)
    nc.vector.memset(eps_t, EPS)
    b1_sb = None
    if b1 is not None:
        b1_sb = const.tile([128, NHID], F32, tag="b1")
        nc.sync.dma_start(out=b1_sb, in_=b1.rearrange("(h p) -> p h", p=128))

    def bcast(src, tag):
        t = const.tile([128, C], F32, tag=tag, name=tag)
        nc.sync.dma_start(
            out=t, in_=bass.AP(tensor=src.tensor, offset=src.offset,
                               ap=[[0, 128]] + list(src.ap)))
        return t

    bo_bc = bcast(bo, "bo") if bo is not None else None
    lnw_bc = bcast(ln2w, "lnw") if general_ln else None
    lnb_bc = bcast(ln2b, "lnb") if general_ln else None
    b2_bc = bcast(b2, "b2") if b2 is not None else None

    xrp = ctx.enter_context(tc.tile_pool(name="xrp", bufs=NRB))
    x2p = ctx.enter_context(tc.tile_pool(name="x2p", bufs=NRB))
    hpool = ctx.enter_context(tc.tile_pool(name="hpool", bufs=2))
    cTp = ctx.enter_context(tc.tile_pool(name="cTp", bufs=1))
    h2Tp = ctx.enter_context(tc.tile_pool(name="h2Tp", bufs=1))
    stat = ctx.enter_context(tc.tile_pool(name="stat", bufs=8))
    wop = ctx.enter_context(tc.tile_pool(name="wop", bufs=1))
    w1p = ctx.enter_context(tc.tile_pool(name="w1p", bufs=3))
    w2p = ctx.enter_context(tc.tile_pool(name="w2p", bufs=1))
    ftp = ctx.enter_context(tc.tile_pool(name="ftp", bufs=NHID))
    osb = ctx.enter_context(tc.tile_pool(name="osb", bufs=2))

    # w2 resident in SBUF; slices streamed inside the W1 loop so the
    # front (catT/wo) and the w1 chunks are not queued behind 8MB
    w2_sb = w2p.tile([128, NHID, C], BF16, tag="w2")

    x2_tiles = []
    # ---- cat arrives pre-transposed [C, RPC]: DMA straight in ----
    catT = cTp.tile([128, NCC, RPC], BF16, tag="catT")
    nc.sync.dma_start(out=catT,
                      in_=cat.rearrange("(cc p) t -> p cc t", p=128))

    # ---- proj (resident Wo, row-outer) + residual + LN2 + transpose,
    # pipelined per row tile ----
    wo_sb = wop.tile([128, NCC, C], BF16, tag="wo")
    wor = wo.rearrange("(cc p) c -> p cc c", p=128)
    nc.gpsimd.dma_start(out=wo_sb[:, 0:4, :], in_=wor[:, 0:4, :])
    nc.gpsimd.dma_start(out=wo_sb[:, 4:8, :], in_=wor[:, 4:8, :])
    h2T = h2Tp.tile([128, NCC, RPC], BF16, tag="h2T")
    xr_tiles = []
    for r in range(NRB):
        xt = xrp.tile([128, C], F32, tag="xr", name=f"xr_{r}")
        nc.sync.dma_start(out=xt, in_=xr[r * 128:(r + 1) * 128, :])
        xr_tiles.append(xt)
    with tc.tile_pool(name="psp", bufs=3, space="PSUM") as PSP, \
         tc.tile_pool(name="pst", bufs=2, space="PSUM") as PST:
        for r in range(NRB):
            pps = PSP.tile([128, C], F32, tag="pp", name=f"pp_{r}")
            for cc in range(NCC):
                for co in range(2):
                    csl = slice(co * 512, (co + 1) * 512)
                    nc.tensor.matmul(pps[:, csl],
                                     catT[:, cc, r * 128:(r + 1) * 128],
                                     wo_sb[:, cc, csl],
                                     start=(cc == 0), stop=(cc == NCC - 1))
            x2t = x2p.tile([128, C], F32, tag="x2", name=f"x2_{r}")
            nc.vector.tensor_add(out=x2t, in0=pps, in1=xr_tiles[r])
            if bo_bc is not None:
                nc.vector.tensor_add(out=x2t, in0=x2t, in1=bo_bc)
            x2_tiles.append(x2t)
            # per-row LN2 chain so h2T streams behind proj
            st = stat.tile([128, 2, 6], F32, tag="bn", name=f"bn_{r}")
            for k in range(2):
                nc.vector.bn_stats(out=st[:, k, :],
                                   in_=x2t[:, k * 512:(k + 1) * 512])
            mv = stat.tile([128, 2], F32, tag="mv", name=f"mv_{r}")
            nc.vector.bn_aggr(out=mv, in_=st)
            sqv = stat.tile([128, 1], F32, tag="sqv", name=f"sqv_{r}")
            nc.scalar.activation(out=sqv, in_=mv[:, 1:2], func=AF.Sqrt,
                                 bias=eps_t)
            rstd = stat.tile([128, 1], F32, tag="rstd", name=f"rstd_{r}")
            nc.vector.reciprocal(out=rstd, in_=sqv)
            ht = hpool.tile([128, C], BF16, tag="h", name=f"h_{r}")
            if general_ln:
                t32 = hpool.tile([128, C], F32, tag="t32", name=f"t32_{r}")
                nc.vector.tensor_scalar(
                    out=t32, in0=x2t, scalar1=mv[:, 0:1], scalar2=rstd,
                    op0=ALU.subtract, op1=ALU.mult)
                nc.vector.tensor_mul(out=t32, in0=t32, in1=lnw_bc)
                nc.vector.tensor_add(out=ht, in0=t32, in1=lnb_bc)
            else:
                nc.vector.tensor_scalar(
                    out=ht, in0=x2t, scalar1=mv[:, 0:1], scalar2=rstd,
                    op0=ALU.subtract, op1=ALU.mult)
            pt = PST.tile([128, C], BF16, tag="tr4", name=f"pt_{r}")
            for cc in range(NCC):
                nc.tensor.transpose(pt[:, cc * 128:(cc + 1) * 128],
                                    ht[:, cc * 128:(cc + 1) * 128], identb)
            nc.vector.tensor_copy(
                out=h2T[:, :, r * 128:(r + 1) * 128],
                in_=pt.rearrange("p (cc q) -> p cc q", q=128))

    with tc.tile_pool(name="psf", bufs=2, space="PSUM") as PSF:
        # ---- phase 1: fT[h] = prelu(W1_h^T @ h2 + b1) ----
        f_tiles = []
        w1r = w1.rearrange("(cc p) (h q) -> p cc h q", p=128, q=128)
        for h in range(NHID):
            w1_sb = w1p.tile([128, NCC, 128], BF16, tag="w1",
                             name=f"w1_{h}")
            nc.sync.dma_start(out=w1_sb, in_=w1r[:, :, h, :])
            nc.gpsimd.dma_start(out=w2_sb[:, h, :],
                                in_=w2[h * 128:(h + 1) * 128, :])
            pf = PSF.tile([128, RPC], F32, tag="ft", name=f"pf_{h}")
            for cc in range(NCC):
                nc.tensor.matmul(pf, w1_sb[:, cc, :], h2T[:, cc, :],
                                 start=(cc == 0), stop=(cc == NCC - 1))
            ft = ftp.tile([128, RPC], BF16, tag="ft", name=f"ft_{h}")
            bias = b1_sb[:, h:h + 1] if b1_sb is not None else 0.0
            nc.scalar.activation(out=ft, in_=pf, func=AF.Prelu,
                                 bias=bias, alpha=alpha)
            f_tiles.append(ft)

    # ---- phase 2: out = fT.T @ W2 (+b2) + x2, row-pair split so the
    # first pair's residual+store overlaps the second pair's matmuls.
    # w2 is SBUF-resident (fetched once at kernel start). ----
    with tc.tile_pool(name="pso", bufs=NRB, space="PSUM") as PSO:
        for rp in range(2):
            rs = (2 * rp, 2 * rp + 1)
            pouts = {r: PSO.tile([128, C], F32, tag="out",
                                 name=f"pout{r}") for r in rs}
            for h in range(NHID):
                for r in rs:
                    for co in range(2):
                        csl = slice(co * 512, (co + 1) * 512)
                        nc.tensor.matmul(
                            pouts[r][:, csl],
                            f_tiles[h][:, r * 128:(r + 1) * 128],
                            w2_sb[:, h, csl],
                            start=(h == 0), stop=(h == NHID - 1))
            for r in rs:
                o_sb = osb.tile([128, C], F32, tag="o", name=f"o_{r}")
                nc.vector.tensor_add(out=o_sb, in0=pouts[r],
                                     in1=x2_tiles[r])
                if b2_bc is not None:
                    nc.vector.tensor_add(out=o_sb, in0=o_sb, in1=b2_bc)
                nc.sync.dma_start(out=out[r * 128:(r + 1) * 128, :],
                                  in_=o_sb)


def _build_ffn(general_ln: bool, has_bo: bool, has_b1: bool, has_b2: bool,
               alpha: float):
    nc = bacc.Bacc("TRN2", target_bir_lowering=False, debug=False)
    xr = nc.dram_tensor("xr", [RPC, C], F32, kind="ExternalInput").ap()
    cat = nc.dram_tensor("cat", [C, RPC], BF16, kind="ExternalInput").ap()
    wo = nc.dram_tensor("wo", [C, C], BF16, kind="ExternalInput").ap()
    w1 = nc.dram_tensor("w1", [C, 4 * C], BF16, kind="ExternalInput").ap()
    w2 = nc.dram_tensor("w2", [4 * C, C], BF16, kind="ExternalInput").ap()
    bo = b1 = ln2w = ln2b = b2 = None
    if has_bo:
        bo = nc.dram_tensor("bo", [C], F32, kind="ExternalInput").ap()
    if has_b1:
        b1 = nc.dram_tensor("b1", [4 * C], F32, kind="ExternalInput").ap()
    if general_ln:
        ln2w = nc.dram_tensor("ln2w", [C], F32, kind="ExternalInput").ap()
        ln2b = nc.dram_tensor("ln2b", [C], F32, kind="ExternalInput").ap()
    if has_b2:
        b2 = nc.dram_tensor("b2", [C], F32, kind="ExternalInput").ap()
    out = nc.dram_tensor("out", [RPC, C], F32, kind="ExternalOutput").ap()
    with tile.TileContext(nc) as tc:
        with ExitStack() as ctx:
            _ffn_body(ctx, tc, xr, cat, wo, w1, w2, bo, b1, ln2w, ln2b, b2,
                      alpha, out)
    nc.compile()
    return nc


# --------------------------------------------------------------------------
# host orchestration
# --------------------------------------------------------------------------

_NC_CACHE = {}


def _get_attn_nc(general_ln):
    key = ("attn", general_ln)
    if key not in _NC_CACHE:
        _NC_CACHE[key] = _build_attn(general_ln)
    return _NC_CACHE[key]


def _get_ffn_nc(general_ln, has_bo, has_b1, has_b2, alpha):
    key = ("ffn", general_ln, has_bo, has_b1, has_b2, float(alpha))
    if key not in _NC_CACHE:
        _NC_CACHE[key] = _build_ffn(general_ln, has_bo, has_b1, has_b2,
                                    float(alpha))
    return _NC_CACHE[key]


def _bf(a):
    return np.ascontiguousarray(np.asarray(a).astype(ml_dtypes.bfloat16))


def attn_in_maps(x_flat, Wq, Wk, Wv, trivial, ln1_w, ln1_b):
    in_maps = []
    for c in range(NCORES):
        b = c // 2
        h0 = (c % 2) * HPC
        m = {
            "x": np.ascontiguousarray(x_flat[b * T:(b + 1) * T]),
            "wq": _bf(np.concatenate([Wq[h] for h in range(h0, h0 + HPC)],
                                     axis=1)),
            "wk": _bf(np.concatenate([Wk[h] for h in range(h0, h0 + HPC)],
                                     axis=1)),
            "wv": _bf(np.concatenate([Wv[h] for h in range(h0, h0 + HPC)],
                                     axis=1)),
        }
        if not trivial:
            m["lnw"] = ln1_w
            m["lnb"] = ln1_b
        in_maps.append(m)
    return in_maps


def assemble_catT(results):
    """Normalize per-core catout [520, T] and assemble catT per batch.

    Returns [B, C, T] bf16: softmax-normalized attention outputs in
    transposed (channel-major) layout, ready for the FFN launch.
    """
    per_batch = []
    for b in range(B):
        halves = []
        for half in range(2):
            raw = np.asarray(results[2 * b + half]["catout"],
                             ml_dtypes.bfloat16).astype(np.float32)
            v = raw.reshape(HPC, 65, T)
            halves.append((v[:, 0:64, :] / v[:, 64:65, :]).reshape(512, T))
        per_batch.append(np.concatenate(halves, axis=0).astype(
            ml_dtypes.bfloat16))
    return np.stack(per_batch, axis=0)


def run_attn(x_flat, Wq, Wk, Wv, ln1_w, ln1_b):
    """Returns catT [B, C, T] bf16: normalized attention outputs."""
    trivial = bool(np.all(ln1_w == 1.0) and np.all(ln1_b == 0.0))
    nc = _get_attn_nc(not trivial)
    in_maps = attn_in_maps(x_flat, Wq, Wk, Wv, trivial, ln1_w, ln1_b)
    res = run_bass_kernel_spmd(nc, in_maps, list(range(NCORES)), trace=False)
    return assemble_catT(res.results)


def ffn_in_maps(x_flat, catT_all, Wo, bo, W1, b1, W2, b2, ln2_w, ln2_b,
                flags):
    """catT_all: [B, C, T] bf16 normalized attention outputs."""
    trivial, has_bo, has_b1, has_b2 = flags
    wo_b = _bf(Wo)
    w1_b = _bf(W1)
    w2_b = _bf(W2)
    if catT_all.dtype != ml_dtypes.bfloat16:
        catT_all = catT_all.astype(ml_dtypes.bfloat16)
    in_maps = []
    for c in range(NCORES):
        sl = slice(RPC * c, RPC * (c + 1))
        b, half = c // 2, c % 2
        m = {
            "xr": np.ascontiguousarray(x_flat[sl]),
            "cat": np.ascontiguousarray(
                catT_all[b][:, half * RPC:(half + 1) * RPC]),
            "wo": wo_b,
            "w1": w1_b,
            "w2": w2_b,
        }
        if has_bo:
            m["bo"] = bo
        if has_b1:
            m["b1"] = b1
        if not trivial:
            m["ln2w"] = ln2_w
            m["ln2b"] = ln2_b
        if has_b2:
            m["b2"] = b2
        in_maps.append(m)
    return in_maps


def run_ffn(x_flat, cat_all, Wo, bo, W1, b1, W2, b2, ln2_w, ln2_b, alpha):
    trivial = bool(np.all(ln2_w == 1.0) and np.all(ln2_b == 0.0))
    has_bo = bool(np.any(bo != 0.0))
    has_b1 = bool(np.any(b1 != 0.0))
    has_b2 = bool(np.any(b2 != 0.0))
    nc = _get_ffn_nc(not trivial, has_bo, has_b1, has_b2, alpha)
    flags = (trivial, has_bo, has_b1, has_b2)
    in_maps = ffn_in_maps(x_flat, cat_all, Wo, bo, W1, b1, W2, b2,
                          ln2_w, ln2_b, flags)
    res = run_bass_kernel_spmd(nc, in_maps, list(range(NCORES)), trace=False)
    return np.concatenate(
        [res.results[c]["out"] for c in range(NCORES)], axis=0)


def kernel(x, ln1_w, ln1_b, Wk, Wq, Wv, Wo, bo, ln2_w, ln2_b, W1, b1,
           prelu_a, W2, b2):
    x = np.asarray(x, np.float32)
    x_flat = np.ascontiguousarray(x.reshape(B * T, C))
    Wq = np.asarray(Wq, np.float32)
    Wk = np.asarray(Wk, np.float32)
    Wv = np.asarray(Wv, np.float32)
    Wo = np.asarray(Wo, np.float32)
    alpha = float(np.asarray(prelu_a))

    cat_all = run_attn(x_flat, Wq, Wk, Wv,
                       np.asarray(ln1_w, np.float32),
                       np.asarray(ln1_b, np.float32))
    out = run_ffn(x_flat, cat_all, Wo, np.asarray(bo, np.float32),
                  np.asarray(W1, np.float32), np.asarray(b1, np.float32),
                  np.asarray(W2, np.float32), np.asarray(b2, np.float32),
                  np.asarray(ln2_w, np.float32),
                  np.asarray(ln2_b, np.float32), alpha)
    return out.reshape(B, T, C).astype(np.float32)
